# revision 6
# baseline (speedup 1.0000x reference)
"""Trainium2 Bass kernel for a BasicTransformerBlock (self-attn + cross-attn + GEGLU FF).

Sharding: 8 cores = 2 batches x 4 sequence chunks of 1024 rows. Each core
redundantly computes LN1 + K/V projections over its batch's full 4096 rows
(position-independent, so all cores run an identical SPMD program) and
produces its own 1024-row slice of the output. No collectives.

Engine plan (sim-model-aware):
- PE p-state stays warm by keeping the matmul stream dense: K/V projection
  matmuls are interleaved as "filler" into the exp-gated attention-1 loop.
- Softmax exp is the dominant ACT/DVE cost (256+ [128,1024] tiles): split
  between ACT (~1.35us/tile) and a DVE poly (~1.5us/tile) by a tuned ratio.
- No ACT table switches mid-attention: LN rstd uses a Newton rsqrt on DVE
  (no Sqrt), all GELUs batch at the end (exp<->gelu table load costs 1.5us).
- attention-1 runs in two 4-head passes so its PV accumulators fit 2 PSUM
  banks (two q-subtiles per bank), leaving a scratch bank pair for
  transposes/projection fillers while scores use 4 banks double-buffered.
- PSUM accumulator banks are initialized by a full-bank zeroing matmul;
  PV matmuls then accumulate with start=False (start_tensor_calc marks the
  whole 2KB bank pending-zero, which would corrupt interleaved regions).
"""

import numpy as np
import ml_dtypes

DIM = 320
HEADS = 8
DH = 40
CTX = 768
IFF = 1280  # GEGLU inner width; proj1 width = 2*IFF
EPS = 1e-5
SCALE = DH ** -0.5
NCORES = 8
MCTX = 77
VS = 336  # V row stride per key block (8*42)

BF16 = ml_dtypes.bfloat16


def _chunks(total, step=128):
    out = []
    k = 0
    while k < total:
        out.append((k, min(step, total - k)))
        k += step
    return out


DIM_CHUNKS = _chunks(DIM)    # [(0,128),(128,128),(256,64)]
CTX_CHUNKS = _chunks(CTX)    # 6 x 128


def _register_exp_op():
    """Custom DVE op: out = (in0*s0 + s1)^32 — exp(z) ~ (1+z/32)^32 so part
    of the softmax exp runs on the vector engine."""
    import concourse.dve_ops as dve_ops
    for o in dve_ops.OPS:
        if o.name == "EXP_POLY32_ANT":
            return o
    from concourse.dve_spec import Spec, Src0, C0, C1, sq
    spec = Spec(
        body=sq(sq(sq(sq(sq(Src0 * C0 + C1))))),
        reference=lambda in0, in1, s0, s1, imm2:
            ((in0.astype(np.float32) * s0 + s1) ** 32).astype(np.float32))
    op = dve_ops.DveOp("EXP_POLY32_ANT", spec, subdim=False,
                       uops_sha={"v3": "eafb894a1d5c531b"})
    dve_ops.OPS.append(op)
    dve_ops._SUB_OPCODE_FOR_NAME[op.name] = \
        dve_ops._CUSTOM_DVE_ROW_BASE + len(dve_ops.OPS) - 1
    dve_ops.CUSTOM_DVE_SPECS[op.name] = op.spec
    return op


def build_nc(S, R, flags=()):
    """Build + compile the per-core Bass program."""
    import contextlib
    from collections import deque
    import concourse.bass as bass
    import concourse.tile as tile
    from concourse import bacc, mybir
    from concourse.masks import make_identity

    f32 = mybir.dt.float32
    bf = mybir.dt.bfloat16
    f8 = mybir.dt.float8e3
    AF = mybir.ActivationFunctionType
    OP = mybir.AluOpType

    flags = set(flags)

    KB = S // 128     # key blocks (self-attn)
    QT = R // 128     # q row-tiles
    NB = S // 512     # 512-token production blocks

    # 3 of 8 softmax-exp tiles go to the DVE poly, the rest to ACT (tunable)
    EXP_DVE_SLOTS = (1, 4, 6)
    ACC_STRIDE = 164  # 4 heads * 41 cols per q-subtile region

    nc = bacc.Bacc("TRN2", target_bir_lowering=False, debug=False)

    def din(name, shape, dt=bf):
        return nc.dram_tensor(name, shape, dt, kind="ExternalInput").ap()

    xfull_d = din("xfull", [S, DIM])
    xq_d = din("xq", [R, DIM], f32)
    ctxT_d = din("ctxT", [CTX, MCTX])
    w_d = {}
    for nm, shape in [
        ("a1_Wq", [DIM, 512]), ("a1_Wk", [DIM, 512]), ("a1_Wv", [DIM, DIM]),
        ("a1_Wo", [DIM, DIM]), ("a2_Wq", [DIM, 512]), ("a2_Wk", [CTX, 512]),
        ("a2_Wv", [CTX, DIM]), ("a2_Wo", [DIM, DIM]),
        ("ff_W1", [DIM, 2 * IFF]), ("ff_W2", [IFF, DIM]),
    ]:
        w_d[nm] = din(nm, shape)
    b1_d = din("ff_b1", [2 * IFF], f32)
    vec_d = {nm: din(nm, [DIM], f32) for nm in sorted(flags)}
    out_d = nc.dram_tensor("out", [R, DIM], f32, kind="ExternalOutput").ap()

    exp_op = _register_exp_op()

    with tile.TileContext(nc) as tc:
        with contextlib.ExitStack() as est:
            persist = est.enter_context(tc.tile_pool(name="persist", bufs=1))
            work = est.enter_context(tc.tile_pool(name="work", bufs=4))
            expp = est.enter_context(tc.tile_pool(name="expp", bufs=5))
            # PSUM budget (8 banks of [128,512]f32):
            #   tag "sc"  : 2 x [128,1024] (4 banks)  attn scores
            #   tag "acc" : 2 x [128,512]  (2 banks)  PV accumulators
            #   tag "scr" : 2 x [128,512]  (2 banks)  transposes/projections
            psum = est.enter_context(tc.tile_pool(name="psum", bufs=2,
                                                  space="PSUM"))

            def ps_sc(shape, dt=f32, name="sc"):
                return psum.tile(shape, dt, tag="sc", bufs=2, name=name)

            def ps_scr(shape, dt=f32, name="scr"):
                return psum.tile(shape, dt, tag="scr", bufs=2, name=name)

            ident = persist.tile([128, 128], bf, name="ident")
            make_identity(nc, ident)
            zrow = persist.tile([1, 512], bf, name="zrow")
            nc.vector.memset(zrow, 0.0)

            # ---- persistent activations
            h1T = persist.tile([128, 3, S], bf, name="h1T")
            Kf = persist.tile([128, 4, S], bf, name="Kf")        # 2-head blocks
            Qf = persist.tile([128, 4, R], bf, name="Qf")
            Vr = persist.tile([128, KB, VS], f8, name="Vr")
            K2f = persist.tile([128, 4, MCTX], bf, name="K2f")   # 2-head blocks
            Q2f = persist.tile([128, 4, 512], bf, name="Q2f")
            V2r = persist.tile([128, VS], f8, name="V2r")
            actT = persist.tile([128, 3, R], bf, name="actT")
            resid = persist.tile([128, QT, DIM], f32, name="resid")
            Uff = persist.tile([128, IFF // 128, R], bf, name="Uff")
            # attention-1 normalized output (row-major), filled per 4-head pass
            arm1 = persist.tile([128, QT, DIM], bf, name="arm1")

            nc.vector.memset(
                Vr[:, :, 0:328].rearrange("p b (h c) -> p b h c",
                                          c=41)[:, :, :, 40], 1.0)

            wsb = {}

            def load_w(names):
                for nm in names:
                    chks = CTX_CHUNKS if nm in ("a2_Wk", "a2_Wv") else DIM_CHUNKS
                    width = w_d[nm].shape[1]
                    t = persist.tile([128, len(chks), width], bf, name=f"w_{nm}",
                                     uniquify=True)
                    for c, (k0, kw) in enumerate(chks):
                        nc.sync.dma_start(out=t[:kw, c, :],
                                          in_=w_d[nm][k0:k0 + kw, :])
                    wsb[nm] = t

            for t in range(QT):
                nc.sync.dma_start(out=resid[:, t, :],
                                  in_=xq_d[t * 128:(t + 1) * 128, :])
            load_w(["a1_Wq", "a1_Wk", "a1_Wv"])

            bcast = {}
            for nm in sorted(flags):
                t = persist.tile([128, DIM], f32, name=f"bc_{nm}")
                src = vec_d[nm]
                bc_ap = bass.AP(tensor=src.tensor, offset=src.offset,
                                ap=[[0, 128]] + [list(p) for p in src.ap])
                nc.gpsimd.dma_start(out=t, in_=bc_ap)
                bcast[nm] = t

            # ---------------- LN via DVE-only rstd (no ACT Sqrt tables) ----
            def ln_batch(srcs, dsts, wkey, bkey, apply_eng=None):
                n = len(srcs)
                mvs = []
                for s in srcs:
                    st = work.tile([128, 6], f32, tag="bnst", bufs=4, name="st")
                    nc.vector.bn_stats(st, s)
                    mv = work.tile([128, 2], f32, tag="bnagg", bufs=8, name="mv")
                    nc.vector.bn_aggr(mv, st)
                    mvs.append(mv)
                v = work.tile([128, n], f32, tag="vbat", bufs=2, name="vbat")
                for i, mv in enumerate(mvs):
                    nc.vector.tensor_scalar(out=v[:, i:i + 1], in0=mv[:, 1:2],
                                            scalar1=EPS, scalar2=None,
                                            op0=OP.add)
                y = work.tile([128, n], f32, tag="ybat", bufs=2, name="ybat")
                nc.vector.tensor_scalar(out=y, in0=v, scalar1=-0.5, scalar2=1.5,
                                        op0=OP.mult, op1=OP.add)
                t1 = work.tile([128, n], f32, tag="tbat", bufs=2, name="tbat")
                for _ in range(3):
                    nc.vector.tensor_tensor(out=t1, in0=y, in1=y, op=OP.mult)
                    nc.vector.tensor_tensor(out=t1, in0=t1, in1=v, op=OP.mult)
                    nc.vector.tensor_scalar(out=t1, in0=t1, scalar1=-0.5,
                                            scalar2=1.5, op0=OP.mult, op1=OP.add)
                    nc.vector.tensor_tensor(out=y, in0=y, in1=t1, op=OP.mult)
                for i, (s, d) in enumerate(zip(srcs, dsts)):
                    eng = (apply_eng or nc.vector)
                    eng.tensor_scalar(
                        out=d, in0=s, scalar1=mvs[i][:, 0:1],
                        scalar2=y[:, i:i + 1], op0=OP.subtract, op1=OP.mult)
                    if wkey in flags:
                        nc.vector.tensor_mul(out=d, in0=d, in1=bcast[wkey])
                    if bkey in flags:
                        nc.vector.tensor_add(out=d, in0=d, in1=bcast[bkey])

            def transpose_batch(dstT, srcs, col0, eng=None):
                """Transpose up to 4 [128,DIM] row-major tiles into dstT
                feature-major columns; per DIM-chunk all tiles share one PSUM
                bank and drain with one wide copy."""
                nt = len(srcs)
                for c, (k0, kw) in enumerate(DIM_CHUNKS):
                    pt = ps_scr([128, 512], bf, name="tr_ps")
                    for i, s in enumerate(srcs):
                        nc.tensor.transpose(pt[:kw, i * 128:(i + 1) * 128],
                                            s[:, k0:k0 + kw], ident)
                    if eng == "act":
                        nc.scalar.activation(dstT[:kw, c, col0:col0 + 128 * nt],
                                             pt[:kw, :128 * nt], AF.Identity)
                    else:
                        nc.vector.tensor_copy(
                            out=dstT[:kw, c, col0:col0 + 128 * nt],
                            in_=pt[:kw, :128 * nt])

            def proj_fm(dst, wt, srcT, n_lo, n_hi, chks, dst_off=0):
                for g in range(4):
                    for n0 in range(n_lo, n_hi, 512):
                        nw = min(512, n_hi - n0)
                        ps = ps_scr([128, 512], name="proj_ps")
                        for c, (k0, kw) in enumerate(chks):
                            nc.tensor.matmul(
                                ps[:, :nw],
                                lhsT=wt[:kw, c, 128 * g:128 * g + 128],
                                rhs=srcT[:kw, c, n0:n0 + nw],
                                start=(c == 0), stop=(c == len(chks) - 1))
                        o0 = dst_off + n0 - n_lo
                        nc.vector.tensor_copy(out=dst[:, g, o0:o0 + nw],
                                              in_=ps[:, :nw])

            def load_late_weights():
                load_w(["a1_Wo", "a2_Wq", "a2_Wk", "a2_Wv", "a2_Wo", "ff_W1"])
                w2 = persist.tile([128, IFF // 128, DIM], bf, name="w_ff2")
                for c in range(IFF // 128):
                    nc.sync.dma_start(out=w2[:, c, :],
                                      in_=w_d["ff_W2"][c * 128:(c + 1) * 128, :])
                b1 = persist.tile([128, (2 * IFF) // 128], f32, name="b1t")
                nc.sync.dma_start(out=b1, in_=b1_d.rearrange("(c p) -> p c", p=128))
                ctxm = persist.tile([128, len(CTX_CHUNKS), MCTX], bf,
                                    name="ctxT_sb")
                for c, (k0, kw) in enumerate(CTX_CHUNKS):
                    nc.sync.dma_start(out=ctxm[:kw, c, :],
                                      in_=ctxT_d[k0:k0 + kw, :])
                return w2, b1, ctxm

            def cross_kv():
                for g in range(4):
                    ps = ps_scr([128, 512], name="k2_ps")
                    for c, (k0, kw) in enumerate(CTX_CHUNKS):
                        nc.tensor.matmul(
                            ps[:, :MCTX],
                            lhsT=wsb["a2_Wk"][:kw, c, 128 * g:128 * g + 128],
                            rhs=ctxT_sb[:kw, c, :],
                            start=(c == 0), stop=(c == len(CTX_CHUNKS) - 1))
                    nc.vector.tensor_copy(out=K2f[:, g, :], in_=ps[:, :MCTX])
                ps = ps_scr([128, 512], name="v2_ps")
                for c, (k0, kw) in enumerate(CTX_CHUNKS):
                    nc.tensor.matmul(
                        ps[:MCTX, :DIM], lhsT=ctxT_sb[:kw, c, :],
                        rhs=wsb["a2_Wv"][:kw, c, :],
                        start=(c == 0), stop=(c == len(CTX_CHUNKS) - 1))
                nc.vector.tensor_copy(
                    out=V2r[:MCTX, 0:328].rearrange("p (h c) -> p h c",
                                                    c=41)[:, :, 0:40],
                    in_=ps[:MCTX, :DIM].rearrange("p (h c) -> p h c", c=40))
                nc.vector.memset(
                    V2r[:MCTX, 0:328].rearrange("p (h c) -> p h c",
                                                c=41)[:, :, 40:41], 1.0)

            # ================= Phase 0: LN1 + transposes ====================
            # Own q rows first (fp32 source) so the Q projection starts early.
            for tbase in range(0, QT, 4):
                srcs = [resid[:, t, :] for t in range(tbase, tbase + 4)]
                hs = [work.tile([128, DIM], bf, tag="h", bufs=8, name="hq")
                      for _ in range(4)]
                ln_batch(srcs, hs, "ln1_w", "ln1_b")
                transpose_batch(actT, hs, tbase * 128)
            proj_fm(Qf, wsb["a1_Wq"], actT, 0, R, DIM_CHUNKS)

            # Full-sequence LN1 -> h1T (feeds K/V projections later)
            for nb in range(NB):
                xts, hs = [], []
                for tt in range(4):
                    t = nb * 4 + tt
                    xt = work.tile([128, DIM], bf, tag="xt", bufs=8, name="xt")
                    nc.sync.dma_start(out=xt,
                                      in_=xfull_d[t * 128:(t + 1) * 128, :])
                    xts.append(xt)
                    hs.append(work.tile([128, DIM], bf, tag="h", bufs=8,
                                        name="h1"))
                ln_batch(xts, hs, "ln1_w", "ln1_b",
                         apply_eng=(nc.gpsimd if tt is not None and nb % 4 == 3
                                    else None))
                transpose_batch(h1T, hs, nb * 512,
                                eng=("act" if nb % 2 == 0 else None))

            # ================= filler factory: K/V production per nb ========
            def kv_fillers(nb):
                units = []

                def k_unit(g):
                    def f():
                        ps = ps_scr([128, 512], name="kf_ps")
                        for c, (k0, kw) in enumerate(DIM_CHUNKS):
                            nc.tensor.matmul(
                                ps,
                                lhsT=wsb["a1_Wk"][:kw, c, 128 * g:128 * g + 128],
                                rhs=h1T[:kw, c, nb * 512:(nb + 1) * 512],
                                start=(c == 0), stop=(c == len(DIM_CHUNKS) - 1))
                        if g % 2 == 0:
                            nc.scalar.activation(
                                Kf[:, g, nb * 512:(nb + 1) * 512], ps,
                                AF.Identity)
                        else:
                            nc.vector.tensor_copy(
                                out=Kf[:, g, nb * 512:(nb + 1) * 512], in_=ps)
                    return f

                def v_unit(t):
                    def f():
                        ps = ps_scr([128, 512], name="v_ps")
                        for c, (k0, kw) in enumerate(DIM_CHUNKS):
                            nc.tensor.matmul(
                                ps[:, :DIM],
                                lhsT=h1T[:kw, c, t * 128:(t + 1) * 128],
                                rhs=wsb["a1_Wv"][:kw, c, :],
                                start=(c == 0), stop=(c == len(DIM_CHUNKS) - 1))
                        nc.scalar.activation(
                            Vr[:, t, 0:328].rearrange("p (h c) -> p h c",
                                                      c=41)[:, :, 0:40],
                            ps[:, :DIM].rearrange("p (h c) -> p h c", c=40),
                            AF.Identity)
                    return f

                for g in range(4):
                    units.append(k_unit(g))
                for tt in range(4):
                    units.append(v_unit(nb * 4 + tt))
                return units

            # ================= attention-1 ==================================
            def zero_bank(t):
                """Full-bank zeroing matmul: clears data AND pending-zero."""
                nc.tensor.matmul(t, lhsT=zrow[0:1, 0:128], rhs=zrow[0:1, :],
                                 start=True, stop=True, skip_group_check=True)

            def attn1_scores_exp(q0, hp, kb, ctr):
                sc = ps_sc([128, 1024], name="sc")
                for j in range(2):
                    hh = 2 * hp + j
                    g, jj = divmod(hh, 2)
                    nc.tensor.matmul(
                        sc[:, j * 512:(j + 1) * 512],
                        lhsT=Kf[64 * jj:64 * jj + 40, g,
                                kb * 128:(kb + 1) * 128],
                        rhs=Qf[64 * jj:64 * jj + 40, g, q0:q0 + 512],
                        start=True, stop=True)
                ep = expp.tile([128, 1024], f8, tag="ep", name="ep")
                if ctr % 8 in EXP_DVE_SLOTS:
                    nc.vector._custom_dve(exp_op, out=ep, in0=sc,
                                          s0=SCALE / 32.0, s1=1.0)
                else:
                    nc.scalar.activation(ep, sc, AF.Exp, scale=SCALE)
                return ep

            def attn1_pv(accs, hp, p, kb, ep):
                for j in range(2):
                    hl = 2 * (hp % 2) + j
                    hh = 2 * hp + j
                    for qs in range(4):
                        bank, sub = divmod(qs, 2)
                        off = sub * ACC_STRIDE + 41 * hl
                        nc.tensor.matmul(
                            accs[bank][:, off:off + 41],
                            lhsT=ep[:, j * 512 + qs * 128:
                                    j * 512 + (qs + 1) * 128],
                            rhs=Vr[:, kb, 41 * hh:41 * hh + 41],
                            start=False, stop=(kb == KB - 1),
                            skip_group_check=True)

            def attn1_pass(qh, p, fillers, ctr):
                """One 4-head pass (head pairs 2p, 2p+1) over all KB key
                blocks, kb-major, interleaving filler units."""
                q0 = qh * 512
                accs = [psum.tile([128, 512], f32, tag="acc", bufs=2,
                                  name=f"acc{qh}{p}{b}") for b in range(2)]
                for b in range(2):
                    zero_bank(accs[b])
                pending = []
                for kb in range(KB):
                    for hp in (2 * p, 2 * p + 1):
                        ep = attn1_scores_exp(q0, hp, kb, ctr)
                        ctr += 1
                        pending.append((hp, kb, ep))
                        if fillers:
                            fillers.popleft()()
                        while len(pending) > 2:
                            hp2, kb2, ep2 = pending.pop(0)
                            attn1_pv(accs, hp2, p, kb2, ep2)
                for hp2, kb2, ep2 in pending:
                    attn1_pv(accs, hp2, p, kb2, ep2)
                # normalize into arm1 row-major bf16
                for qs in range(4):
                    bank, sub = divmod(qs, 2)
                    reg = accs[bank][:, sub * ACC_STRIDE:sub * ACC_STRIDE + 164]
                    rec = work.tile([128, 4], f32, tag="rec", bufs=4, name="rec")
                    nc.vector.reciprocal(
                        rec, reg.rearrange("p (h c) -> p h c", c=41)[:, :, 40])
                    rb = bass.AP(tensor=rec.tensor, offset=rec.offset,
                                 ap=[list(rec.ap[0]), [rec.ap[1][0], 4],
                                     [0, 40]])
                    t = qh * 4 + qs
                    nc.vector.tensor_mul(
                        out=arm1[:, t, p * 160:(p + 1) * 160].rearrange(
                            "p (h c) -> p h c", c=40),
                        in0=reg.rearrange("p (h c) -> p h c",
                                          c=41)[:, :, 0:40],
                        in1=rb)
                return ctr

            def attn1_out(qh):
                for qs in range(4):
                    t = qh * 4 + qs
                    afm = work.tile([128, 3, 128], bf, tag="afm", name="afm")
                    for c, (k0, kw) in enumerate(DIM_CHUNKS):
                        pt = ps_scr([128, 512], bf, name="afm_ps")
                        nc.tensor.transpose(pt[:kw, :128],
                                            arm1[:, t, k0:k0 + kw], ident)
                        nc.vector.tensor_copy(out=afm[:kw, c, :],
                                              in_=pt[:kw, :128])
                    po = ps_scr([128, 512], name="po")
                    for c, (k0, kw) in enumerate(DIM_CHUNKS):
                        nc.tensor.matmul(po[:, :DIM], lhsT=afm[:kw, c, :],
                                         rhs=wsb["a1_Wo"][:kw, c, :],
                                         start=(c == 0),
                                         stop=(c == len(DIM_CHUNKS) - 1))
                    nc.vector.tensor_add(out=resid[:, t, :],
                                         in0=resid[:, t, :], in1=po[:, :DIM])
                    if "a1_bo" in flags:
                        nc.vector.tensor_add(out=resid[:, t, :],
                                             in0=resid[:, t, :],
                                             in1=bcast["a1_bo"])

            # ================= cross-attention ==============================
            def attn2(qh):
                srcs = [resid[:, qh * 4 + tt, :] for tt in range(4)]
                hs = [work.tile([128, DIM], bf, tag="h", bufs=8, name="h2")
                      for _ in range(4)]
                ln_batch(srcs, hs, "ln2_w", "ln2_b")
                transpose_batch(actT, hs, 0)
                proj_fm(Q2f, wsb["a2_Wq"], actT, 0, 512, DIM_CHUNKS)

                p2 = []
                for hp in range(4):
                    sc = ps_sc([128, 1024], name="sc2")
                    for j in range(2):
                        hh = 2 * hp + j
                        g, jj = divmod(hh, 2)
                        nc.tensor.matmul(
                            sc[:MCTX, j * 512:(j + 1) * 512],
                            lhsT=K2f[64 * jj:64 * jj + 40, g, :],
                            rhs=Q2f[64 * jj:64 * jj + 40, g, 0:512],
                            start=True, stop=True)
                    ep = expp.tile([128, 1024], f8, tag="ep2", bufs=4,
                                   name="ep2")
                    nc.scalar.activation(ep[:MCTX, :], sc[:MCTX, :], AF.Exp,
                                         scale=SCALE)
                    p2.append((hp, ep))

                armq = [work.tile([128, DIM], bf, tag="arm2", bufs=4,
                                  name=f"arm2_{qs}") for qs in range(4)]
                for p in range(2):
                    accs = [psum.tile([128, 512], f32, tag="acc", bufs=2,
                                      name=f"a2c{qh}{p}{b}") for b in range(2)]
                    for b in range(2):
                        zero_bank(accs[b])
                    for hp, ep in p2[2 * p:2 * p + 2]:
                        for j in range(2):
                            hl = 2 * (hp % 2) + j
                            hh = 2 * hp + j
                            for qs in range(4):
                                bank, sub = divmod(qs, 2)
                                off = sub * ACC_STRIDE + 41 * hl
                                nc.tensor.matmul(
                                    accs[bank][:, off:off + 41],
                                    lhsT=ep[:MCTX, j * 512 + qs * 128:
                                            j * 512 + (qs + 1) * 128],
                                    rhs=V2r[:MCTX, 41 * hh:41 * hh + 41],
                                    start=False, stop=True,
                                    skip_group_check=True)
                    for qs in range(4):
                        bank, sub = divmod(qs, 2)
                        reg = accs[bank][:, sub * ACC_STRIDE:
                                         sub * ACC_STRIDE + 164]
                        rec = work.tile([128, 4], f32, tag="rec", bufs=4,
                                        name="rec2")
                        nc.vector.reciprocal(
                            rec, reg.rearrange("p (h c) -> p h c",
                                               c=41)[:, :, 40])
                        rb = bass.AP(tensor=rec.tensor, offset=rec.offset,
                                     ap=[list(rec.ap[0]), [rec.ap[1][0], 4],
                                         [0, 40]])
                        nc.vector.tensor_mul(
                            out=armq[qs][:, p * 160:(p + 1) * 160].rearrange(
                                "p (h c) -> p h c", c=40),
                            in0=reg.rearrange("p (h c) -> p h c",
                                              c=41)[:, :, 0:40],
                            in1=rb)
                for qs in range(4):
                    t = qh * 4 + qs
                    afm = work.tile([128, 3, 128], bf, tag="afm", name="afm2")
                    for c, (k0, kw) in enumerate(DIM_CHUNKS):
                        pt = ps_scr([128, 512], bf, name="afm2_ps")
                        nc.tensor.transpose(pt[:kw, :128],
                                            armq[qs][:, k0:k0 + kw], ident)
                        nc.vector.tensor_copy(out=afm[:kw, c, :],
                                              in_=pt[:kw, :128])
                    po = ps_scr([128, 512], name="po2")
                    for c, (k0, kw) in enumerate(DIM_CHUNKS):
                        nc.tensor.matmul(po[:, :DIM], lhsT=afm[:kw, c, :],
                                         rhs=wsb["a2_Wo"][:kw, c, :],
                                         start=(c == 0),
                                         stop=(c == len(DIM_CHUNKS) - 1))
                    nc.vector.tensor_add(out=resid[:, t, :],
                                         in0=resid[:, t, :], in1=po[:, :DIM])
                    if "a2_bo" in flags:
                        nc.vector.tensor_add(out=resid[:, t, :],
                                             in0=resid[:, t, :],
                                             in1=bcast["a2_bo"])

            # ================= driver =======================================
            fillers = deque()
            for nb in range(NB):
                fillers.extend(kv_fillers(nb))
            # nb0's K/V must exist before the first QK
            for _ in range(8):
                fillers.popleft()()
            w2_sb, b1t, ctxT_sb = load_late_weights()
            fillers.append(cross_kv)

            ctr = 0
            ctr = attn1_pass(0, 0, fillers, ctr)
            ctr = attn1_pass(0, 1, fillers, ctr)
            while fillers:
                fillers.popleft()()
            attn1_out(0)
            attn2(0)
            ctr = attn1_pass(1, 0, fillers, ctr)
            ctr = attn1_pass(1, 1, fillers, ctr)
            attn1_out(1)
            attn2(1)

            # ================= GEGLU feed-forward (both halves) =============
            NMT = (2 * IFF) // 128  # 20
            for qh in range(2):
                srcs = [resid[:, qh * 4 + tt, :] for tt in range(4)]
                hs = [work.tile([128, DIM], bf, tag="h", bufs=8, name="h3")
                      for _ in range(4)]
                ln_batch(srcs, hs, "ln3_w", "ln3_b")
                transpose_batch(actT, hs, 512 * qh)
            for qh in range(2):
                q0 = qh * 512
                _order = [m for pair in zip(range(NMT // 2),
                                            range(NMT // 2, NMT))
                          for m in pair]
                for mt in _order:
                    ps = ps_scr([128, 512], name="ff1_ps")
                    for c, (k0, kw) in enumerate(DIM_CHUNKS):
                        nc.tensor.matmul(
                            ps,
                            lhsT=wsb["ff_W1"][:kw, c, mt * 128:(mt + 1) * 128],
                            rhs=actT[:kw, c, q0:q0 + 512],
                            start=(c == 0), stop=(c == len(DIM_CHUNKS) - 1))
                    if mt < NMT // 2:
                        nc.vector.tensor_scalar(
                            out=Uff[:, mt, q0:q0 + 512], in0=ps,
                            scalar1=b1t[:, mt:mt + 1], scalar2=None,
                            op0=OP.add)
                    else:
                        gl = work.tile([128, 512], bf, tag="gel", name="gel")
                        nc.scalar.activation(gl, ps, AF.Gelu,
                                             bias=b1t[:, mt:mt + 1], scale=1.0)
                        mu = mt - NMT // 2
                        nc.vector.tensor_mul(out=Uff[:, mu, q0:q0 + 512],
                                             in0=Uff[:, mu, q0:q0 + 512],
                                             in1=gl)
                for tt in range(4):
                    qs = qh * 4 + tt
                    po = ps_scr([128, 512], name="ff2_ps")
                    for c in range(IFF // 128):
                        nc.tensor.matmul(po[:, :DIM],
                                         lhsT=Uff[:, c, qs * 128:(qs + 1) * 128],
                                         rhs=w2_sb[:, c, :],
                                         start=(c == 0),
                                         stop=(c == IFF // 128 - 1))
                    ot = work.tile([128, DIM], f32, tag="ot", name="ot")
                    nc.vector.tensor_add(out=ot, in0=resid[:, qs, :],
                                         in1=po[:, :DIM])
                    if "ff_b2" in flags:
                        nc.vector.tensor_add(out=ot, in0=ot, in1=bcast["ff_b2"])
                    nc.sync.dma_start(out=out_d[qs * 128:(qs + 1) * 128, :],
                                      in_=ot)

    nc.compile()
    return nc


_CACHE = {}


def _get_nc(S, R, flags):
    key = (S, R, tuple(sorted(flags)))
    if key not in _CACHE:
        _CACHE[key] = build_nc(S, R, flags)
    return _CACHE[key]


def _pad_qk2(w):
    """Q/K layout: 2-head groups at partition offsets {0,64}."""
    w = np.asarray(w)
    out = np.zeros((w.shape[0], 512), w.dtype)
    for h in range(HEADS):
        g, j = divmod(h, 2)
        out[:, 128 * g + 64 * j:128 * g + 64 * j + DH] = w[:, DH * h:DH * h + DH]
    return out


def make_in_maps(x, context, ln_params, weights):
    """Host-side prep: returns (flags, in_maps, R, S, Bn)."""
    x = np.asarray(x)
    context = np.asarray(context)
    Bn = x.shape[0]
    S = x.shape[1]
    R = S * Bn // NCORES
    flags = set()
    for nm in ("ln1_w", "ln2_w", "ln3_w"):
        if not np.allclose(np.asarray(ln_params[nm]), 1.0):
            flags.add(nm)
    for nm in ("ln1_b", "ln2_b", "ln3_b", "a1_bo", "a2_bo", "ff_b2"):
        if not np.allclose(np.asarray(ln_params[nm]), 0.0):
            flags.add(nm)
    weights = dict(weights)
    weights["a1_Wq"] = _pad_qk2(weights["a1_Wq"])
    weights["a1_Wk"] = _pad_qk2(weights["a1_Wk"])
    weights["a2_Wq"] = _pad_qk2(weights["a2_Wq"])
    weights["a2_Wk"] = _pad_qk2(weights["a2_Wk"])
    shared = {nm: np.ascontiguousarray(np.asarray(w).astype(BF16))
              for nm, w in weights.items()}
    shared["ff_b1"] = np.ascontiguousarray(
        np.asarray(ln_params["ff_b1"]).astype(np.float32))
    for nm in flags:
        shared[nm] = np.ascontiguousarray(
            np.asarray(ln_params[nm]).astype(np.float32))
    xbf = np.ascontiguousarray(x.astype(BF16))
    ctxT = np.ascontiguousarray(np.asarray(context).astype(BF16).transpose(0, 2, 1))
    xf32 = np.ascontiguousarray(x.astype(np.float32))
    in_maps = []
    cpb = NCORES // Bn
    for core in range(NCORES):
        b, c = divmod(core, cpb)
        m = dict(shared)
        m["xfull"] = xbf[b]
        m["xq"] = np.ascontiguousarray(xf32[b, c * R:(c + 1) * R])
        m["ctxT"] = ctxT[b]
        in_maps.append(m)
    return flags, in_maps, R, S, Bn


def kernel(x, context, ln1_w, ln1_b, ln2_w, ln2_b, ln3_w, ln3_b,
           a1_Wq, a1_Wk, a1_Wv, a1_Wo, a1_bo,
           a2_Wq, a2_Wk, a2_Wv, a2_Wo, a2_bo,
           ff_W1, ff_b1, ff_W2, ff_b2, _trace=False):
    from concourse.bass_utils import run_bass_kernel_spmd

    weights = dict(a1_Wq=a1_Wq, a1_Wk=a1_Wk, a1_Wv=a1_Wv, a1_Wo=a1_Wo,
                   a2_Wq=a2_Wq, a2_Wk=a2_Wk, a2_Wv=a2_Wv, a2_Wo=a2_Wo,
                   ff_W1=ff_W1, ff_W2=ff_W2)
    ln_params = dict(ln1_w=ln1_w, ln1_b=ln1_b, ln2_w=ln2_w, ln2_b=ln2_b,
                     ln3_w=ln3_w, ln3_b=ln3_b, a1_bo=a1_bo, a2_bo=a2_bo,
                     ff_b1=ff_b1, ff_b2=ff_b2)
    flags, in_maps, R, S, Bn = make_in_maps(x, context, ln_params, weights)
    nc = _get_nc(S, R, flags)
    res = run_bass_kernel_spmd(nc, in_maps, core_ids=list(range(NCORES)),
                               trace=_trace)
    out = np.empty((Bn, S, DIM), np.float32)
    cpb = NCORES // Bn
    for core in range(NCORES):
        b, c = divmod(core, cpb)
        out[b, c * R:(c + 1) * R] = res.results[core]["out"]
    kernel._last_result = res
    return out
